# revision 38
# baseline (speedup 1.0000x reference)
"""Trainium2 Bass kernel for nn_DifferentiableSimulator.

Strategy (8 NeuronCores, B=8): one batch element per core, no collectives.

Host side (cheap, O(V+N)):
  - per-batch probe geometry: rotation, LUT bilinear interp (tiny)
  - per-batch voxel relevance sharding: keep voxels within CUT(10mm) +
    probe-radius of the shank axis segment.  Dropped voxels have weights
    < e^-32 relative to any weight that can influence an output pixel
    (full argument in the repo notes); empirically the output matches the
    dense reference to ~2e-4.  Keeps ~1.3k of 10k voxels.
  - lattice factorization: the 1000 contacts are a rigid 10x10x10 grid,
    so in the rotated frame  d2[n,v] = (x_i-wx_v)^2 + (y_j-wy_v)^2 +
    (z_k-wz_v)^2  with w = R^T (v - grid_center).  The soft-match weight
    matrix factorizes as W[n,v] = Wxy[(ij),v] * Wz[k,v]: only 110 gaussian
    columns per voxel instead of 1000.  Host ships the voxel features
    (fp16 hi/lo pairs so the fp16 matmul is ~fp32-exact: fp16 products are
    exact in the fp32 PSUM accumulator) and the 138 lattice columns.
  - contacts are reindexed m = k*128 + (iy*10+ix)  (28 dummy xy slots per
    z-layer with weight 0) so the per-z-layer weighted sums land exactly
    in contact-chunk layout with no transposes.

Device side (per core), phase 1 -- soft PRF match per 128-voxel chunk:
  one K=17 fp16 matmul -> xy/z gaussian exponents [128v, 138] in PSUM;
  ACT exp -> [Wxy | Wz] fp32; one DVE op forms WzE = Wz x [pol, ecc, 1]
  (broadcast APs); one fp32 matmul accumulates B[128ij, 30] =
  sum_v Wxy^T (Wz*E)  = all weighted sums, already contact-major.

Phase 2 -- splat:
  The reference's per-electrode 256x256 gaussian splat is separable:
  exp(-((gx-cx)^2+(gy-cy)^2)/s^2) = col_factor * row_factor, and
  rot90(map) just swaps/mirrors the centers (row center 255-cx, col
  center cy).  Per-contact params (sin/cos via the hardware Sin spline); row/col factors
  via ACT Square with scale=1/s folded in, one big exp per 4/4/2 chunks;
  20 fp16 matmuls accumulate the 256x256 map; global max via the
  PE-transpose trick; scale; DMA out.

A PE warmup burst runs during startup to coax the HAM clock gate to
2.4 GHz (unreliable on these cores; the kernel is sized to be fast even
at the cold 1.2 GHz PE clock).
"""
import math
from contextlib import ExitStack

import numpy as np

import concourse.bass as bass
import concourse.mybir as mybir
from concourse import tile
from concourse.bass_utils import run_bass_kernel_spmd

# ---- constants (must match the reference) ----
_CMAG_A = 0.75
_CMAG_B = 120.0
_CMAG_K = 17.3
_DEG2RAD = math.pi / 180.0
AMP = 100.0
_SPREAD = math.sqrt(AMP / 675.0)
VIEW_ANGLE = 90.0
MAP_SIZE = 256
SOFT_MATCH_SIGMA = 1.5

B = 8
NCC = 10                  # contact chunks = z-layers
NXY = 128                 # xy-lattice slots per layer (100 real + 28 dummy)
CUT = 10.0
XY_RAD = 1.8 * math.sqrt(2.0)
SE = MAP_SIZE / VIEW_ANGLE
KSIG = _SPREAD / 2.0 * SE
EXP_SCALE = 2.0 / (2.0 * SOFT_MATCH_SIGMA ** 2)   # 2/4.5

# sin(y) ~ y*(c0 + c1 y^2 + ... + c4 y^8) on [-pi, pi]; max err 1.7e-5
SIN_C = (9.99984590e-01, -1.66632589e-01, 8.31238590e-03,
         -1.93162309e-04, 2.17323611e-06)

f32 = mybir.dt.float32
f16 = mybir.dt.float16
i32 = mybir.dt.int32
AF = mybir.ActivationFunctionType
ALU = mybir.AluOpType
PI = math.pi


# ---------------------------------------------------------------- host prep
def _f16s(x):
    hi = np.float16(x)
    lo = np.float16(np.float32(x) - np.float32(hi))
    return hi, lo


def _f16_split(x):
    hi = x.astype(np.float16)
    lo = (x.astype(np.float32) - hi.astype(np.float32)).astype(np.float16)
    return hi.astype(np.float32), lo.astype(np.float32)


def _host_geometry(params, start_loc, surf_dist_lut, alpha_grid, beta_grid):
    params = params.astype(np.float64)
    alpha, beta, offset, shank = (params[:, 0], params[:, 1],
                                  params[:, 2], params[:, 3])
    a = alpha * _DEG2RAD
    b = beta * _DEG2RAD
    ca, sa = np.cos(a), np.sin(a)
    cb, sb = np.cos(b), np.sin(b)
    Bn = params.shape[0]
    Rx = np.zeros((Bn, 3, 3)); Ry = np.zeros((Bn, 3, 3))
    Rx[:, 0, 0] = 1; Rx[:, 1, 1] = ca; Rx[:, 1, 2] = -sa
    Rx[:, 2, 1] = sa; Rx[:, 2, 2] = ca
    Ry[:, 0, 0] = cb; Ry[:, 0, 2] = sb; Ry[:, 1, 1] = 1
    Ry[:, 2, 0] = -sb; Ry[:, 2, 2] = cb
    R = Rx @ Ry
    direction = np.einsum('bij,j->bi', R, np.array([0.0, 0.0, -1.0]))
    direction = direction / np.linalg.norm(direction, axis=-1, keepdims=True)
    lut = surf_dist_lut.astype(np.float64)
    na, nb = lut.shape
    ag, bg = alpha_grid.astype(np.float64), beta_grid.astype(np.float64)
    a_norm = 2.0 * (alpha - ag[0]) / (ag[-1] - ag[0] + 1e-08) - 1.0
    b_norm = 2.0 * (beta - bg[0]) / (bg[-1] - bg[0] + 1e-08) - 1.0
    ai = np.clip((a_norm + 1.0) * 0.5 * (na - 1), 0.0, na - 1.0)
    bi = np.clip((b_norm + 1.0) * 0.5 * (nb - 1), 0.0, nb - 1.0)
    a0 = np.clip(np.floor(ai), 0, na - 1).astype(np.int64)
    b0 = np.clip(np.floor(bi), 0, nb - 1).astype(np.int64)
    a1 = np.minimum(a0 + 1, na - 1)
    b1 = np.minimum(b0 + 1, nb - 1)
    fa = ai - a0
    fb = bi - b0
    v00 = lut[a0, b0]; v01 = lut[a0, b1]; v10 = lut[a1, b0]; v11 = lut[a1, b1]
    surf = (v00 * (1 - fa) * (1 - fb) + v01 * (1 - fa) * fb
            + v10 * fa * (1 - fb) + v11 * fa * fb)
    surf = np.maximum(surf, 1.0)
    penetration = surf - shank / 2.0 - offset
    grid_center = (start_loc.astype(np.float64)[None, :]
                   + direction * penetration[:, None])
    return grid_center, R, direction, shank


def _voxel_keep(v1_pos, grid_center, axis_dir, half_len):
    d = v1_pos.astype(np.float64) - grid_center[None, :]
    t = np.clip(d @ axis_dir, -half_len, half_len)
    dist = np.linalg.norm(d - t[:, None] * axis_dir[None, :], axis=1)
    return dist <= (CUT + XY_RAD + 0.5)


def _prep_core(gc_b, R_b, shank_b, logits_b, v1_pos_k, v1_prf_k, VP):
    """Per-core device input arrays for the lattice-factorized kernel."""
    Vk = v1_pos_k.shape[0]
    w = np.zeros((VP, 3))
    w[:Vk] = (v1_pos_k.astype(np.float64) - gc_b[None, :]) @ R_b
    wf = w.astype(np.float32)
    wh, wl = _f16_split(wf)
    bxy = (-0.5 * (w[:, 0] ** 2 + w[:, 1] ** 2)).astype(np.float32)
    bz = (-0.5 * w[:, 2] ** 2).astype(np.float32)
    bxy[Vk:] = -30000.0
    bz[Vk:] = -30000.0
    bxyh, bxyl = _f16_split(bxy)
    bzh, bzl = _f16_split(bz)
    onesv = np.ones(VP, np.float32)
    vt = np.stack([wh[:, 0], wh[:, 1], wl[:, 0], wl[:, 1], wh[:, 0],
                   wh[:, 1], onesv, onesv, bxyh, bxyl,
                   wh[:, 2], wl[:, 2], wh[:, 2], onesv, onesv, bzh, bzl],
                  axis=0).astype(np.float16)

    xs = np.arange(10) * 0.4 - 1.8
    zs = (np.linspace(0.0, 1.0, 10) - 0.5) * float(shank_b)
    cols = np.zeros((17, NXY + 10), np.float32)
    for ij in range(NXY):
        if ij < 100:
            iy, ix = ij // 10, ij % 10
            x, y = xs[ix], xs[iy]
            xh, xl = _f16s(x)
            yh, yl = _f16s(y)
            axyh, axyl = _f16s(-0.5 * (x * x + y * y))
            cols[0:10, ij] = [xh, yh, xh, yh, xl, yl, axyh, axyl, 1.0, 1.0]
        else:
            cols[6, ij] = -30000.0     # dummy xy slot -> Wxy = 0
            cols[8, ij] = 1.0
    for k in range(10):
        z = zs[k]
        zh, zl = _f16s(z)
        azh, azl = _f16s(-0.5 * z * z)
        cols[10:17, NXY + k] = [zh, zh, zl, azh, azl, 1.0, 1.0]
    rhs = cols.astype(np.float16)

    nch = VP // 128
    e3 = np.zeros((VP, 3), np.float32)
    e3[:Vk, 0] = v1_prf_k[:, 0]
    e3[:Vk, 1] = v1_prf_k[:, 1]
    e3[:Vk, 2] = 1.0
    e3t = np.ascontiguousarray(
        e3.reshape(nch, 128, 3).transpose(1, 0, 2).reshape(128, 3 * nch))

    lgt = np.full((NXY, NCC), -30.0, np.float32)
    iy, ix = np.divmod(np.arange(100), 10)
    for k in range(NCC):
        lgt[:100, k] = logits_b[iy * 100 + ix * 10 + k]
    return {"vt": vt, "rhs": rhs, "e3": e3t,
            "lgt": np.ascontiguousarray(lgt),
            "eye": np.eye(128, dtype=np.float32)}


# ------------------------------------------------------------- device kernel
def _split_multiwaits(nc):
    """This walrus build accepts at most ONE sync wait per instruction.
    Tile emits several.  Engine instruction streams execute in order, so
    moving all but one wait onto single-wait NoOps inserted just before
    the instruction preserves semantics exactly."""
    cnt = 0
    for fn in nc.m.functions:
        for blk in fn.blocks:
            out = []
            for inst in blk.instructions:
                si = inst.sync_info
                if si is not None and si.on_wait is not None \
                        and len(si.on_wait) > 1:
                    waits = list(si.on_wait)
                    for w in waits[:-1]:
                        cnt += 1
                        out.append(mybir.InstNoOp(
                            name=f"WSPLIT-{cnt}",
                            engine=inst.engine,
                            ins=[], outs=[],
                            sync_info=mybir.SyncInfo(on_wait=[w],
                                                     on_update=[]),
                        ))
                    inst.sync_info = mybir.SyncInfo(
                        on_wait=[waits[-1]], on_update=list(si.on_update))
                out.append(inst)
            blk.instructions = out
    return cnt


def _build_nc(VP):
    nch = VP // 128
    NL = NXY + 10    # 138 lattice columns
    nc = bass.Bass()
    vt_d = nc.dram_tensor("vt", [17, VP], f16, kind="ExternalInput")
    rhs_d = nc.dram_tensor("rhs", [17, NL], f16, kind="ExternalInput")
    e3_d = nc.dram_tensor("e3", [128, 3 * nch], f32, kind="ExternalInput")
    lgt_d = nc.dram_tensor("lgt", [NXY, NCC], f32, kind="ExternalInput")
    eye_d = nc.dram_tensor("eye", [128, 128], f32, kind="ExternalInput")
    out_d = nc.dram_tensor("out", [MAP_SIZE, MAP_SIZE], f32,
                           kind="ExternalOutput")

    with ExitStack() as ctx:
        tc = ctx.enter_context(tile.TileContext(nc))
        constp = ctx.enter_context(tc.tile_pool(name="const", bufs=1))
        parm = ctx.enter_context(tc.tile_pool(name="parm", bufs=1))
        work = ctx.enter_context(tc.tile_pool(name="work", bufs=3))
        psA = ctx.enter_context(
            tc.tile_pool(name="psA", bufs=1, space=bass.MemorySpace.PSUM))

        # Warmups first (top scheduler priority): ACT table load + PE HAM
        # burst run during the sem-init + input-DMA window.
        scr = constp.tile([1, 1], f32, tag="scr", name="scr")
        nc.vector.memset(scr[:], 0.0)
        nc.scalar.activation(scr[:], scr[:], AF.Exp, bias=0.0, scale=1.0)
        wrm = constp.tile([128, 512], f16, tag="wrm", name="wrm")
        nc.vector.memset(wrm[:], 0.0)
        wps = psA.tile([128, 512], f32, tag="wps", name="wps")
        for _ in range(12):
            nc.tensor.matmul(wps[:], wrm[:, 0:128], wrm[:],
                             start=True, stop=True, skip_group_check=True)

        vt_t = constp.tile([17, VP], f16, tag="vt", name="vt")
        nc.sync.dma_start(vt_t[:], vt_d[:])
        rhs_t = constp.tile([17, NL], f16, tag="rhs", name="rhs")
        nc.gpsimd.dma_start(rhs_t[:], rhs_d[:])
        e3_t = constp.tile([128, 3 * nch], f32, tag="e3", name="e3")
        nc.scalar.dma_start(e3_t[:], e3_d[:])
        lg_t = constp.tile([NXY, NCC], f32, tag="lgt", name="lgt")
        nc.sync.dma_start(lg_t[:], lgt_d[:])
        eye_t = constp.tile([128, 128], f32, tag="eye", name="eye")
        nc.gpsimd.dma_start(eye_t[:], eye_d[:])

        ii_t = constp.tile([128, MAP_SIZE], i32, tag="ii", name="ii")
        nc.gpsimd.iota(ii_t[:], pattern=[[1, MAP_SIZE]], base=0,
                       channel_multiplier=0)
        iof = constp.tile([128, MAP_SIZE], f32, tag="iof", name="iof")
        nc.vector.tensor_copy(iof[:], ii_t[:])
        ones_t = constp.tile([1, 128], f32, tag="ones", name="ones")
        nc.vector.memset(ones_t[:], 1.0)

        # sigmoid(logits) is independent of phase 1 -- run it early on the
        # otherwise-idle DVE.
        en = parm.tile([128, NCC], f32, tag="en", name="en")
        nc.scalar.activation(en[:], lg_t[:], AF.Exp, bias=0.0, scale=-1.0)
        nc.vector.tensor_scalar_add(en[:], en[:], 1.0)
        pb = parm.tile([128, NCC], f32, tag="pb", name="pb")
        nc.vector.reciprocal(pb[:], en[:])

        # ---------------- phase 1: factorized soft match ----------------
        B_ps = psA.tile([128, 3 * NCC], f32, tag="B", name="B")
        with tc.tile_pool(name="psW", bufs=3,
                          space=bass.MemorySpace.PSUM) as psW:
            for k in range(nch):
                ct = psW.tile([128, NL], f32, tag="cross", name="cross")
                nc.tensor.matmul(ct[:], vt_t[:, k * 128:(k + 1) * 128],
                                 rhs_t[:], start=True, stop=True)
                wx = work.tile([128, NL], f32, tag="wx", name="wx")
                nc.scalar.activation(wx[:], ct[:], AF.Exp,
                                     bias=0.0, scale=EXP_SCALE)
                wze = work.tile([128, 3 * NCC], f32, tag="wze", name="wze")
                e3b = e3_t[:, 3 * k:3 * k + 3] \
                    .rearrange("p (one f) -> p one f", one=1) \
                    .broadcast_to([128, NCC, 3])
                wzb = wx[:, NXY:NL] \
                    .rearrange("p (k one) -> p k one", one=1) \
                    .broadcast_to([128, NCC, 3])
                nc.vector.tensor_tensor(
                    wze[:].rearrange("p (k f) -> p k f", f=3),
                    e3b, wzb, ALU.mult)
                nc.tensor.matmul(B_ps[:], wx[:, 0:NXY], wze[:],
                                 start=(k == 0), stop=(k == nch - 1))

        bsb = parm.tile([128, 3 * NCC], f32, tag="bsb", name="bsb")
        nc.vector.tensor_copy(bsb[:], B_ps[:])
        bs3 = bsb[:].rearrange("p (k f) -> p k f", f=3)

        with tc.tile_pool(name="psM", bufs=1,
                          space=bass.MemorySpace.PSUM) as psM:
            def pt(tag):
                return parm.tile([128, NCC], f32, tag=tag, name=tag)

            # ---------------- per-contact params ----------------
            t0 = pt("t0")
            nc.vector.tensor_scalar_add(t0[:], bs3[:, :, 2], 1e-8)
            rws = pt("rws"); nc.vector.reciprocal(rws[:], t0[:])
            pol = pt("pol")
            nc.vector.tensor_mul(pol[:], bs3[:, :, 0], rws[:])
            ecc = pt("ecc")
            nc.vector.tensor_mul(ecc[:], bs3[:, :, 1], rws[:])

            # t20 = [theta | pi/2 - |theta|]; ACT Sin of both halves gives
            # sin(phi) and cos(phi) (hardware spline, 4-ULP).
            t20 = parm.tile([128, 2 * NCC], f32, tag="t20", name="t20")
            nc.vector.tensor_scalar(t20[:, 0:NCC], pol[:], _DEG2RAD, -PI,
                                    ALU.mult, ALU.add)
            nc.vector.tensor_scalar(t20[:, 0:NCC], t20[:, 0:NCC], PI, -PI,
                                    ALU.min, ALU.max)
            nc.scalar.activation(t20[:, NCC:2 * NCC], t20[:, 0:NCC], AF.Abs)
            nc.vector.tensor_scalar(t20[:, NCC:2 * NCC], t20[:, NCC:2 * NCC],
                                    -1.0, PI / 2.0, ALU.mult, ALU.add)
            sc20 = parm.tile([128, 2 * NCC], f32, tag="sc20", name="sc20")
            nc.scalar.activation(sc20[:], t20[:], AF.Sin)
            # dummy exp right after the Sin: pulls the exp-table reload into
            # the params window instead of stalling ACT before the first
            # phase-2 mega-exp (Square runs fine in the exp set).
            dummy_exp = nc.scalar.activation(scr[:], scr[:], AF.Exp,
                                             bias=0.0, scale=1.0)
            sn = sc20[:, 0:NCC]
            cs = sc20[:, NCC:2 * NCC]

            t1 = pt("t1"); nc.vector.tensor_mul(t1[:], ecc[:], cs)
            t2 = pt("t2"); nc.vector.tensor_mul(t2[:], ecc[:], sn)
            nyc = pt("nyc")
            nc.vector.tensor_scalar(nyc[:], t1[:], -SE, -127.0,
                                    ALU.mult, ALU.add)
            nxc = pt("nxc")
            nc.vector.tensor_scalar(nxc[:], t2[:], SE, -128.0,
                                    ALU.mult, ALU.add)

            q1 = pt("q1"); nc.vector.tensor_scalar_add(q1[:], ecc[:], _CMAG_A)
            r1 = pt("r1"); nc.vector.reciprocal(r1[:], q1[:])
            q2 = pt("q2"); nc.vector.tensor_scalar_add(q2[:], ecc[:], _CMAG_B)
            r2 = pt("r2"); nc.vector.reciprocal(r2[:], q2[:])
            dd = pt("dd"); nc.vector.tensor_sub(dd[:], r1[:], r2[:])
            nc.vector.tensor_scalar(dd[:], dd[:], _CMAG_K, 1e-8,
                                    ALU.mult, ALU.add)
            mi = pt("mi"); nc.vector.reciprocal(mi[:], dd[:])
            ps = pt("ps")
            nc.vector.tensor_scalar(ps[:], mi[:], KSIG, 1.0,
                                    ALU.mult, ALU.max)
            sr = pt("sr"); nc.vector.reciprocal(sr[:], ps[:])
            sbx = pt("sbx"); nc.vector.tensor_mul(sbx[:], sr[:], nxc[:])
            sby = pt("sby"); nc.vector.tensor_mul(sby[:], sr[:], nyc[:])

            val = pt("val")
            nc.vector.tensor_scalar_min(val[:], bs3[:, :, 2], 1.0)
            wc = pt("wc"); nc.vector.tensor_mul(wc[:], pb[:], val[:])

            # ---------------- phase 2: separable splat ----------------
            mp0 = psM.tile([128, MAP_SIZE], f32, tag="map0", name="map0")
            mp1 = psM.tile([128, MAP_SIZE], f32, tag="map1", name="map1")
            GROUPS = [(0, 4), (4, 4), (8, 2)]
            for g0, GRP in GROUPS:
                sq4 = work.tile([128, 4 * 2 * MAP_SIZE], f32, tag="sq4",
                                name="sq4")
                for q in range(GRP):
                    c = g0 + q
                    o = q * 2 * MAP_SIZE
                    nc.scalar.activation(sq4[:, o:o + MAP_SIZE], iof[:],
                                         AF.Square, bias=sby[:, c:c + 1],
                                         scale=sr[:, c:c + 1])
                    xs = work.tile([128, MAP_SIZE], f32, tag="xs", name="xs")
                    nc.vector.tensor_scalar(xs[:], iof[:], sr[:, c:c + 1],
                                            sbx[:, c:c + 1], ALU.mult,
                                            ALU.add)
                    nc.vector.tensor_mul(
                        sq4[:, o + MAP_SIZE:o + 2 * MAP_SIZE], xs[:], xs[:])
                xy8 = work.tile([128, 4 * 2 * MAP_SIZE], f16, tag="xy8",
                                name="xy8")
                nc.scalar.activation(xy8[:, 0:GRP * 2 * MAP_SIZE],
                                     sq4[:, 0:GRP * 2 * MAP_SIZE],
                                     AF.Exp, bias=0.0, scale=-1.0)
                for q in range(GRP):
                    c = g0 + q
                    o = q * 2 * MAP_SIZE
                    yy = work.tile([128, MAP_SIZE], f16, tag="yy", name="yy")
                    nc.vector.tensor_scalar_mul(yy[:], xy8[:, o:o + MAP_SIZE],
                                                wc[:, c:c + 1])
                    xx = xy8[:, o + MAP_SIZE:o + 2 * MAP_SIZE]
                    nc.tensor.matmul(mp0[:], yy[:, 0:128], xx,
                                     start=(c == 0), stop=(c == NCC - 1))
                    nc.tensor.matmul(mp1[:], yy[:, 128:256], xx,
                                     start=(c == 0), stop=(c == NCC - 1))

            # ---------------- normalize + store ----------------
            m0 = parm.tile([128, 1], f32, tag="m0", name="m0")
            nc.vector.reduce_max(m0[:], mp0[:], axis=mybir.AxisListType.X)
            m1 = parm.tile([128, 1], f32, tag="m1", name="m1")
            nc.vector.reduce_max(m1[:], mp1[:], axis=mybir.AxisListType.X)
            mx = parm.tile([128, 1], f32, tag="mx", name="mx")
            nc.vector.tensor_max(mx[:], m0[:], m1[:])
            mt = psM.tile([1, 128], f32, tag="mt", name="mt")
            nc.tensor.transpose(mt[:], mx[:], eye_t[:, :])
            gm = parm.tile([1, 1], f32, tag="gm", name="gm")
            nc.vector.reduce_max(gm[:], mt[:], axis=mybir.AxisListType.X)
            nc.vector.tensor_scalar_add(gm[:], gm[:], 1e-8)
            gi = parm.tile([1, 1], f32, tag="gi", name="gi")
            nc.vector.reciprocal(gi[:], gm[:])
            gb = psM.tile([128, 1], f32, tag="gb", name="gb")
            nc.tensor.matmul(gb[:], ones_t[:], gi[:], start=True, stop=True)
            gs = parm.tile([128, 1], f32, tag="gs", name="gs")
            nc.vector.tensor_copy(gs[:], gb[:])

            o0 = work.tile([128, MAP_SIZE], f32, tag="o0", name="o0")
            nc.vector.tensor_scalar_mul(o0[:], mp0[:], gs[:])
            o1 = work.tile([128, MAP_SIZE], f32, tag="o1", name="o1")
            nc.scalar.activation(o1[:], mp1[:], AF.Copy, scale=gs[:])
            nc.sync.dma_start(out_d[0:128, :], o0[:])
            nc.scalar.dma_start(out_d[128:256, :], o1[:])
    return nc


# ----------------------------------------------------------------- entry
def _run(inputs, trace=False):
    params = np.asarray(inputs["params"], np.float32)
    logits = np.asarray(inputs["electrode_logits"], np.float32)
    v1_pos = np.asarray(inputs["v1_pos"], np.float32)
    v1_prf = np.asarray(inputs["v1_prf"], np.float32)
    start_loc = np.asarray(inputs["start_loc"], np.float32)
    surf_dist_lut = np.asarray(inputs["surf_dist_lut"], np.float32)
    alpha_grid = np.asarray(inputs["alpha_grid"], np.float32)
    beta_grid = np.asarray(inputs["beta_grid"], np.float32)

    gc, R, direction, shank = _host_geometry(
        params, start_loc, surf_dist_lut, alpha_grid, beta_grid)
    keeps = [_voxel_keep(v1_pos, gc[b], R[b, :, 2], shank[b] / 2.0)
             for b in range(B)]
    nkeep = max(int(k.sum()) for k in keeps)
    VP = max(128, ((nkeep + 127) // 128) * 128)

    in_maps = []
    for b in range(B):
        k = keeps[b]
        in_maps.append(_prep_core(gc[b], R[b], shank[b], logits[b],
                                  v1_pos[k], v1_prf[k], VP))
    nc = _build_nc(VP)
    _split_multiwaits(nc)
    res = run_bass_kernel_spmd(nc, in_maps, list(range(B)), trace=trace)
    out = np.stack([res.results[i]["out"] for i in range(B)])
    return out[:, None, :, :].astype(np.float32), res


def kernel(**inputs) -> np.ndarray:
    out, _ = _run(inputs, trace=False)
    return out


# revision 39
# speedup vs baseline: 1.0101x; 1.0101x over previous
"""Trainium2 Bass kernel for nn_DifferentiableSimulator.

Strategy (8 NeuronCores, B=8): one batch element per core, no collectives.

Host side (cheap, O(V+N)):
  - per-batch probe geometry: rotation, LUT bilinear interp (tiny)
  - per-batch voxel relevance sharding: keep voxels within CUT(10mm) +
    probe-radius of the shank axis segment.  Dropped voxels have weights
    < e^-32 relative to any weight that can influence an output pixel
    (full argument in the repo notes); empirically the output matches the
    dense reference to ~2e-4.  Keeps ~1.3k of 10k voxels.
  - lattice factorization: the 1000 contacts are a rigid 10x10x10 grid,
    so in the rotated frame  d2[n,v] = (x_i-wx_v)^2 + (y_j-wy_v)^2 +
    (z_k-wz_v)^2  with w = R^T (v - grid_center).  The soft-match weight
    matrix factorizes as W[n,v] = Wxy[(ij),v] * Wz[k,v]: only 110 gaussian
    columns per voxel instead of 1000.  Host ships the voxel features
    (fp16 hi/lo pairs so the fp16 matmul is ~fp32-exact: fp16 products are
    exact in the fp32 PSUM accumulator) and the 138 lattice columns.
  - contacts are reindexed m = k*128 + (iy*10+ix)  (28 dummy xy slots per
    z-layer with weight 0) so the per-z-layer weighted sums land exactly
    in contact-chunk layout with no transposes.

Device side (per core), phase 1 -- soft PRF match per 128-voxel chunk:
  one K=17 fp16 matmul -> xy/z gaussian exponents [128v, 138] in PSUM;
  ACT exp -> [Wxy | Wz] fp32; one DVE op forms WzE = Wz x [pol, ecc, 1]
  (broadcast APs); one fp32 matmul accumulates B[128ij, 30] =
  sum_v Wxy^T (Wz*E)  = all weighted sums, already contact-major.

Phase 2 -- splat:
  The reference's per-electrode 256x256 gaussian splat is separable:
  exp(-((gx-cx)^2+(gy-cy)^2)/s^2) = col_factor * row_factor, and
  rot90(map) just swaps/mirrors the centers (row center 255-cx, col
  center cy).  Per-contact params (sin/cos via the hardware Sin spline); row/col factors
  via ACT Square with scale=1/s folded in, one big exp per 4/4/2 chunks;
  20 fp16 matmuls accumulate the 256x256 map; global max via the
  PE-transpose trick; scale; DMA out.

A PE warmup burst runs during startup to coax the HAM clock gate to
2.4 GHz (unreliable on these cores; the kernel is sized to be fast even
at the cold 1.2 GHz PE clock).
"""
import math
from contextlib import ExitStack

import numpy as np

import concourse.bass as bass
import concourse.mybir as mybir
from concourse import tile
from concourse.bass_utils import run_bass_kernel_spmd

# ---- constants (must match the reference) ----
_CMAG_A = 0.75
_CMAG_B = 120.0
_CMAG_K = 17.3
_DEG2RAD = math.pi / 180.0
AMP = 100.0
_SPREAD = math.sqrt(AMP / 675.0)
VIEW_ANGLE = 90.0
MAP_SIZE = 256
SOFT_MATCH_SIGMA = 1.5

B = 8
NCC = 10                  # contact chunks = z-layers
NXY = 128                 # xy-lattice slots per layer (100 real + 28 dummy)
CUT = 10.0
XY_RAD = 1.8 * math.sqrt(2.0)
SE = MAP_SIZE / VIEW_ANGLE
KSIG = _SPREAD / 2.0 * SE
EXP_SCALE = 2.0 / (2.0 * SOFT_MATCH_SIGMA ** 2)   # 2/4.5

# sin(y) ~ y*(c0 + c1 y^2 + ... + c4 y^8) on [-pi, pi]; max err 1.7e-5
SIN_C = (9.99984590e-01, -1.66632589e-01, 8.31238590e-03,
         -1.93162309e-04, 2.17323611e-06)

f32 = mybir.dt.float32
f16 = mybir.dt.float16
i32 = mybir.dt.int32
AF = mybir.ActivationFunctionType
ALU = mybir.AluOpType
PI = math.pi


# ---------------------------------------------------------------- host prep
def _f16s(x):
    hi = np.float16(x)
    lo = np.float16(np.float32(x) - np.float32(hi))
    return hi, lo


def _f16_split(x):
    hi = x.astype(np.float16)
    lo = (x.astype(np.float32) - hi.astype(np.float32)).astype(np.float16)
    return hi.astype(np.float32), lo.astype(np.float32)


def _host_geometry(params, start_loc, surf_dist_lut, alpha_grid, beta_grid):
    params = params.astype(np.float64)
    alpha, beta, offset, shank = (params[:, 0], params[:, 1],
                                  params[:, 2], params[:, 3])
    a = alpha * _DEG2RAD
    b = beta * _DEG2RAD
    ca, sa = np.cos(a), np.sin(a)
    cb, sb = np.cos(b), np.sin(b)
    Bn = params.shape[0]
    Rx = np.zeros((Bn, 3, 3)); Ry = np.zeros((Bn, 3, 3))
    Rx[:, 0, 0] = 1; Rx[:, 1, 1] = ca; Rx[:, 1, 2] = -sa
    Rx[:, 2, 1] = sa; Rx[:, 2, 2] = ca
    Ry[:, 0, 0] = cb; Ry[:, 0, 2] = sb; Ry[:, 1, 1] = 1
    Ry[:, 2, 0] = -sb; Ry[:, 2, 2] = cb
    R = Rx @ Ry
    direction = np.einsum('bij,j->bi', R, np.array([0.0, 0.0, -1.0]))
    direction = direction / np.linalg.norm(direction, axis=-1, keepdims=True)
    lut = surf_dist_lut.astype(np.float64)
    na, nb = lut.shape
    ag, bg = alpha_grid.astype(np.float64), beta_grid.astype(np.float64)
    a_norm = 2.0 * (alpha - ag[0]) / (ag[-1] - ag[0] + 1e-08) - 1.0
    b_norm = 2.0 * (beta - bg[0]) / (bg[-1] - bg[0] + 1e-08) - 1.0
    ai = np.clip((a_norm + 1.0) * 0.5 * (na - 1), 0.0, na - 1.0)
    bi = np.clip((b_norm + 1.0) * 0.5 * (nb - 1), 0.0, nb - 1.0)
    a0 = np.clip(np.floor(ai), 0, na - 1).astype(np.int64)
    b0 = np.clip(np.floor(bi), 0, nb - 1).astype(np.int64)
    a1 = np.minimum(a0 + 1, na - 1)
    b1 = np.minimum(b0 + 1, nb - 1)
    fa = ai - a0
    fb = bi - b0
    v00 = lut[a0, b0]; v01 = lut[a0, b1]; v10 = lut[a1, b0]; v11 = lut[a1, b1]
    surf = (v00 * (1 - fa) * (1 - fb) + v01 * (1 - fa) * fb
            + v10 * fa * (1 - fb) + v11 * fa * fb)
    surf = np.maximum(surf, 1.0)
    penetration = surf - shank / 2.0 - offset
    grid_center = (start_loc.astype(np.float64)[None, :]
                   + direction * penetration[:, None])
    return grid_center, R, direction, shank


def _voxel_keep(v1_pos, grid_center, axis_dir, half_len):
    d = v1_pos.astype(np.float64) - grid_center[None, :]
    t = np.clip(d @ axis_dir, -half_len, half_len)
    dist = np.linalg.norm(d - t[:, None] * axis_dir[None, :], axis=1)
    return dist <= (CUT + XY_RAD + 0.5)


def _prep_core(gc_b, R_b, shank_b, logits_b, v1_pos_k, v1_prf_k, VP):
    """Per-core device input arrays for the lattice-factorized kernel."""
    Vk = v1_pos_k.shape[0]
    w = np.zeros((VP, 3))
    w[:Vk] = (v1_pos_k.astype(np.float64) - gc_b[None, :]) @ R_b
    wf = w.astype(np.float32)
    wh, wl = _f16_split(wf)
    bxy = (-0.5 * (w[:, 0] ** 2 + w[:, 1] ** 2)).astype(np.float32)
    bz = (-0.5 * w[:, 2] ** 2).astype(np.float32)
    bxy[Vk:] = -30000.0
    bz[Vk:] = -30000.0
    bxyh, bxyl = _f16_split(bxy)
    bzh, bzl = _f16_split(bz)
    onesv = np.ones(VP, np.float32)
    vt = np.stack([wh[:, 0], wh[:, 1], wl[:, 0], wl[:, 1], wh[:, 0],
                   wh[:, 1], onesv, onesv, bxyh, bxyl,
                   wh[:, 2], wl[:, 2], wh[:, 2], onesv, onesv, bzh, bzl],
                  axis=0).astype(np.float16)

    xs = np.arange(10) * 0.4 - 1.8
    zs = (np.linspace(0.0, 1.0, 10) - 0.5) * float(shank_b)
    cols = np.zeros((17, NXY + 10), np.float32)
    for ij in range(NXY):
        if ij < 100:
            iy, ix = ij // 10, ij % 10
            x, y = xs[ix], xs[iy]
            xh, xl = _f16s(x)
            yh, yl = _f16s(y)
            axyh, axyl = _f16s(-0.5 * (x * x + y * y))
            cols[0:10, ij] = [xh, yh, xh, yh, xl, yl, axyh, axyl, 1.0, 1.0]
        else:
            cols[6, ij] = -30000.0     # dummy xy slot -> Wxy = 0
            cols[8, ij] = 1.0
    for k in range(10):
        z = zs[k]
        zh, zl = _f16s(z)
        azh, azl = _f16s(-0.5 * z * z)
        cols[10:17, NXY + k] = [zh, zh, zl, azh, azl, 1.0, 1.0]
    rhs = cols.astype(np.float16)

    nch = VP // 128
    e3 = np.zeros((VP, 3), np.float32)
    e3[:Vk, 0] = v1_prf_k[:, 0]
    e3[:Vk, 1] = v1_prf_k[:, 1]
    e3[:Vk, 2] = 1.0
    e3t = np.ascontiguousarray(
        e3.reshape(nch, 128, 3).transpose(1, 0, 2).reshape(128, 3 * nch))

    lgt = np.full((NXY, NCC), -30.0, np.float32)
    iy, ix = np.divmod(np.arange(100), 10)
    for k in range(NCC):
        lgt[:100, k] = logits_b[iy * 100 + ix * 10 + k]
    return {"vt": vt, "rhs": rhs, "e3": e3t,
            "lgt": np.ascontiguousarray(lgt),
            "eye": np.eye(128, dtype=np.float32)}


# ------------------------------------------------------------- device kernel
def _split_multiwaits(nc):
    """This walrus build accepts at most ONE sync wait per instruction.
    Tile emits several.  Engine instruction streams execute in order, so
    moving all but one wait onto single-wait NoOps inserted just before
    the instruction preserves semantics exactly."""
    cnt = 0
    for fn in nc.m.functions:
        for blk in fn.blocks:
            out = []
            for inst in blk.instructions:
                si = inst.sync_info
                if si is not None and si.on_wait is not None \
                        and len(si.on_wait) > 1:
                    waits = list(si.on_wait)
                    for w in waits[:-1]:
                        cnt += 1
                        out.append(mybir.InstNoOp(
                            name=f"WSPLIT-{cnt}",
                            engine=inst.engine,
                            ins=[], outs=[],
                            sync_info=mybir.SyncInfo(on_wait=[w],
                                                     on_update=[]),
                        ))
                    inst.sync_info = mybir.SyncInfo(
                        on_wait=[waits[-1]], on_update=list(si.on_update))
                out.append(inst)
            blk.instructions = out
    return cnt


def _build_nc(VP):
    nch = VP // 128
    NL = NXY + 10    # 138 lattice columns
    nc = bass.Bass()
    vt_d = nc.dram_tensor("vt", [17, VP], f16, kind="ExternalInput")
    rhs_d = nc.dram_tensor("rhs", [17, NL], f16, kind="ExternalInput")
    e3_d = nc.dram_tensor("e3", [128, 3 * nch], f32, kind="ExternalInput")
    lgt_d = nc.dram_tensor("lgt", [NXY, NCC], f32, kind="ExternalInput")
    eye_d = nc.dram_tensor("eye", [128, 128], f32, kind="ExternalInput")
    out_d = nc.dram_tensor("out", [MAP_SIZE, MAP_SIZE], f32,
                           kind="ExternalOutput")

    with ExitStack() as ctx:
        tc = ctx.enter_context(tile.TileContext(nc))
        constp = ctx.enter_context(tc.tile_pool(name="const", bufs=1))
        parm = ctx.enter_context(tc.tile_pool(name="parm", bufs=1))
        work = ctx.enter_context(tc.tile_pool(name="work", bufs=4))
        psA = ctx.enter_context(
            tc.tile_pool(name="psA", bufs=1, space=bass.MemorySpace.PSUM))

        # Warmups first (top scheduler priority): ACT table load + PE HAM
        # burst run during the sem-init + input-DMA window.
        scr = constp.tile([1, 1], f32, tag="scr", name="scr")
        nc.vector.memset(scr[:], 0.0)
        nc.scalar.activation(scr[:], scr[:], AF.Exp, bias=0.0, scale=1.0)
        wrm = constp.tile([128, 512], f16, tag="wrm", name="wrm")
        nc.vector.memset(wrm[:], 0.0)
        wps = psA.tile([128, 512], f32, tag="wps", name="wps")
        for _ in range(12):
            nc.tensor.matmul(wps[:], wrm[:, 0:128], wrm[:],
                             start=True, stop=True, skip_group_check=True)

        vt_t = constp.tile([17, VP], f16, tag="vt", name="vt")
        nc.sync.dma_start(vt_t[:], vt_d[:])
        rhs_t = constp.tile([17, NL], f16, tag="rhs", name="rhs")
        nc.gpsimd.dma_start(rhs_t[:], rhs_d[:])
        e3_t = constp.tile([128, 3 * nch], f32, tag="e3", name="e3")
        nc.scalar.dma_start(e3_t[:], e3_d[:])
        lg_t = constp.tile([NXY, NCC], f32, tag="lgt", name="lgt")
        nc.sync.dma_start(lg_t[:], lgt_d[:])
        eye_t = constp.tile([128, 128], f32, tag="eye", name="eye")
        nc.gpsimd.dma_start(eye_t[:], eye_d[:])

        ii_t = constp.tile([128, MAP_SIZE], i32, tag="ii", name="ii")
        nc.gpsimd.iota(ii_t[:], pattern=[[1, MAP_SIZE]], base=0,
                       channel_multiplier=0)
        iof = constp.tile([128, MAP_SIZE], f32, tag="iof", name="iof")
        nc.vector.tensor_copy(iof[:], ii_t[:])
        ones_t = constp.tile([1, 128], f32, tag="ones", name="ones")
        nc.vector.memset(ones_t[:], 1.0)

        # sigmoid(logits) is independent of phase 1 -- run it early on the
        # otherwise-idle DVE.
        en = parm.tile([128, NCC], f32, tag="en", name="en")
        nc.scalar.activation(en[:], lg_t[:], AF.Exp, bias=0.0, scale=-1.0)
        nc.vector.tensor_scalar_add(en[:], en[:], 1.0)
        pb = parm.tile([128, NCC], f32, tag="pb", name="pb")
        nc.vector.reciprocal(pb[:], en[:])

        # ---------------- phase 1: factorized soft match ----------------
        B_ps = psA.tile([128, 3 * NCC], f32, tag="B", name="B")
        with tc.tile_pool(name="psW", bufs=3,
                          space=bass.MemorySpace.PSUM) as psW:
            for k in range(nch):
                ct = psW.tile([128, NL], f32, tag="cross", name="cross")
                nc.tensor.matmul(ct[:], vt_t[:, k * 128:(k + 1) * 128],
                                 rhs_t[:], start=True, stop=True)
                wx = work.tile([128, NL], f32, tag="wx", name="wx")
                nc.scalar.activation(wx[:], ct[:], AF.Exp,
                                     bias=0.0, scale=EXP_SCALE)
                wze = work.tile([128, 3 * NCC], f32, tag="wze", name="wze")
                e3b = e3_t[:, 3 * k:3 * k + 3] \
                    .rearrange("p (one f) -> p one f", one=1) \
                    .broadcast_to([128, NCC, 3])
                wzb = wx[:, NXY:NL] \
                    .rearrange("p (k one) -> p k one", one=1) \
                    .broadcast_to([128, NCC, 3])
                nc.vector.tensor_tensor(
                    wze[:].rearrange("p (k f) -> p k f", f=3),
                    e3b, wzb, ALU.mult)
                nc.tensor.matmul(B_ps[:], wx[:, 0:NXY], wze[:],
                                 start=(k == 0), stop=(k == nch - 1))

        bsb = parm.tile([128, 3 * NCC], f32, tag="bsb", name="bsb")
        nc.vector.tensor_copy(bsb[:], B_ps[:])
        bs3 = bsb[:].rearrange("p (k f) -> p k f", f=3)

        with tc.tile_pool(name="psM", bufs=1,
                          space=bass.MemorySpace.PSUM) as psM:
            def pt(tag):
                return parm.tile([128, NCC], f32, tag=tag, name=tag)

            # ---------------- per-contact params ----------------
            t0 = pt("t0")
            nc.vector.tensor_scalar_add(t0[:], bs3[:, :, 2], 1e-8)
            rws = pt("rws"); nc.vector.reciprocal(rws[:], t0[:])
            pol = pt("pol")
            nc.vector.tensor_mul(pol[:], bs3[:, :, 0], rws[:])
            ecc = pt("ecc")
            nc.vector.tensor_mul(ecc[:], bs3[:, :, 1], rws[:])

            # t20 = [theta | pi/2 - |theta|]; ACT Sin of both halves gives
            # sin(phi) and cos(phi) (hardware spline, 4-ULP).
            t20 = parm.tile([128, 2 * NCC], f32, tag="t20", name="t20")
            nc.vector.tensor_scalar(t20[:, 0:NCC], pol[:], _DEG2RAD, -PI,
                                    ALU.mult, ALU.add)
            nc.vector.tensor_scalar(t20[:, 0:NCC], t20[:, 0:NCC], PI, -PI,
                                    ALU.min, ALU.max)
            nc.scalar.activation(t20[:, NCC:2 * NCC], t20[:, 0:NCC], AF.Abs)
            nc.vector.tensor_scalar(t20[:, NCC:2 * NCC], t20[:, NCC:2 * NCC],
                                    -1.0, PI / 2.0, ALU.mult, ALU.add)
            sc20 = parm.tile([128, 2 * NCC], f32, tag="sc20", name="sc20")
            nc.scalar.activation(sc20[:], t20[:], AF.Sin)
            # dummy exp right after the Sin: pulls the exp-table reload into
            # the params window instead of stalling ACT before the first
            # phase-2 mega-exp (Square runs fine in the exp set).
            dummy_exp = nc.scalar.activation(scr[:], scr[:], AF.Exp,
                                             bias=0.0, scale=1.0)
            sn = sc20[:, 0:NCC]
            cs = sc20[:, NCC:2 * NCC]

            t1 = pt("t1"); nc.vector.tensor_mul(t1[:], ecc[:], cs)
            t2 = pt("t2"); nc.vector.tensor_mul(t2[:], ecc[:], sn)
            nyc = pt("nyc")
            nc.vector.tensor_scalar(nyc[:], t1[:], -SE, -127.0,
                                    ALU.mult, ALU.add)
            nxc = pt("nxc")
            nc.vector.tensor_scalar(nxc[:], t2[:], SE, -128.0,
                                    ALU.mult, ALU.add)

            q1 = pt("q1"); nc.vector.tensor_scalar_add(q1[:], ecc[:], _CMAG_A)
            r1 = pt("r1"); nc.vector.reciprocal(r1[:], q1[:])
            q2 = pt("q2"); nc.vector.tensor_scalar_add(q2[:], ecc[:], _CMAG_B)
            r2 = pt("r2"); nc.vector.reciprocal(r2[:], q2[:])
            dd = pt("dd"); nc.vector.tensor_sub(dd[:], r1[:], r2[:])
            nc.vector.tensor_scalar(dd[:], dd[:], _CMAG_K, 1e-8,
                                    ALU.mult, ALU.add)
            mi = pt("mi"); nc.vector.reciprocal(mi[:], dd[:])
            ps = pt("ps")
            nc.vector.tensor_scalar(ps[:], mi[:], KSIG, 1.0,
                                    ALU.mult, ALU.max)
            sr = pt("sr"); nc.vector.reciprocal(sr[:], ps[:])
            sbx = pt("sbx"); nc.vector.tensor_mul(sbx[:], sr[:], nxc[:])
            sby = pt("sby"); nc.vector.tensor_mul(sby[:], sr[:], nyc[:])

            val = pt("val")
            nc.vector.tensor_scalar_min(val[:], bs3[:, :, 2], 1.0)
            wc = pt("wc"); nc.vector.tensor_mul(wc[:], pb[:], val[:])

            # ---------------- phase 2: separable splat ----------------
            mp0 = psM.tile([128, MAP_SIZE], f32, tag="map0", name="map0")
            mp1 = psM.tile([128, MAP_SIZE], f32, tag="map1", name="map1")
            GROUPS = [(0, 4), (4, 4), (8, 2)]
            for g0, GRP in GROUPS:
                sq4 = work.tile([128, 4 * 2 * MAP_SIZE], f32, tag="sq4",
                                name="sq4")
                for q in range(GRP):
                    c = g0 + q
                    o = q * 2 * MAP_SIZE
                    nc.scalar.activation(sq4[:, o:o + MAP_SIZE], iof[:],
                                         AF.Square, bias=sby[:, c:c + 1],
                                         scale=sr[:, c:c + 1])
                    xs = work.tile([128, MAP_SIZE], f32, tag="xs", name="xs")
                    nc.vector.tensor_scalar(xs[:], iof[:], sr[:, c:c + 1],
                                            sbx[:, c:c + 1], ALU.mult,
                                            ALU.add)
                    nc.vector.tensor_mul(
                        sq4[:, o + MAP_SIZE:o + 2 * MAP_SIZE], xs[:], xs[:])
                xy8 = work.tile([128, 4 * 2 * MAP_SIZE], f16, tag="xy8",
                                name="xy8")
                nc.scalar.activation(xy8[:, 0:GRP * 2 * MAP_SIZE],
                                     sq4[:, 0:GRP * 2 * MAP_SIZE],
                                     AF.Exp, bias=0.0, scale=-1.0)
                for q in range(GRP):
                    c = g0 + q
                    o = q * 2 * MAP_SIZE
                    yy = work.tile([128, MAP_SIZE], f16, tag="yy", name="yy")
                    nc.vector.tensor_scalar_mul(yy[:], xy8[:, o:o + MAP_SIZE],
                                                wc[:, c:c + 1])
                    xx = xy8[:, o + MAP_SIZE:o + 2 * MAP_SIZE]
                    nc.tensor.matmul(mp0[:], yy[:, 0:128], xx,
                                     start=(c == 0), stop=(c == NCC - 1))
                    nc.tensor.matmul(mp1[:], yy[:, 128:256], xx,
                                     start=(c == 0), stop=(c == NCC - 1))

            # ---------------- normalize + store ----------------
            m0 = parm.tile([128, 1], f32, tag="m0", name="m0")
            nc.vector.reduce_max(m0[:], mp0[:], axis=mybir.AxisListType.X)
            m1 = parm.tile([128, 1], f32, tag="m1", name="m1")
            nc.vector.reduce_max(m1[:], mp1[:], axis=mybir.AxisListType.X)
            mx = parm.tile([128, 1], f32, tag="mx", name="mx")
            nc.vector.tensor_max(mx[:], m0[:], m1[:])
            mt = psM.tile([1, 128], f32, tag="mt", name="mt")
            nc.tensor.transpose(mt[:], mx[:], eye_t[:, :])
            gm = parm.tile([1, 1], f32, tag="gm", name="gm")
            nc.vector.reduce_max(gm[:], mt[:], axis=mybir.AxisListType.X)
            nc.vector.tensor_scalar_add(gm[:], gm[:], 1e-8)
            gi = parm.tile([1, 1], f32, tag="gi", name="gi")
            nc.vector.reciprocal(gi[:], gm[:])
            gb = psM.tile([128, 1], f32, tag="gb", name="gb")
            nc.tensor.matmul(gb[:], ones_t[:], gi[:], start=True, stop=True)
            gs = parm.tile([128, 1], f32, tag="gs", name="gs")
            nc.vector.tensor_copy(gs[:], gb[:])

            o0 = work.tile([128, MAP_SIZE], f32, tag="o0", name="o0")
            nc.vector.tensor_scalar_mul(o0[:], mp0[:], gs[:])
            o1 = work.tile([128, MAP_SIZE], f32, tag="o1", name="o1")
            nc.scalar.activation(o1[:], mp1[:], AF.Copy, scale=gs[:])
            nc.sync.dma_start(out_d[0:128, :], o0[:])
            nc.scalar.dma_start(out_d[128:256, :], o1[:])
    return nc


# ----------------------------------------------------------------- entry
def _run(inputs, trace=False):
    params = np.asarray(inputs["params"], np.float32)
    logits = np.asarray(inputs["electrode_logits"], np.float32)
    v1_pos = np.asarray(inputs["v1_pos"], np.float32)
    v1_prf = np.asarray(inputs["v1_prf"], np.float32)
    start_loc = np.asarray(inputs["start_loc"], np.float32)
    surf_dist_lut = np.asarray(inputs["surf_dist_lut"], np.float32)
    alpha_grid = np.asarray(inputs["alpha_grid"], np.float32)
    beta_grid = np.asarray(inputs["beta_grid"], np.float32)

    gc, R, direction, shank = _host_geometry(
        params, start_loc, surf_dist_lut, alpha_grid, beta_grid)
    keeps = [_voxel_keep(v1_pos, gc[b], R[b, :, 2], shank[b] / 2.0)
             for b in range(B)]
    nkeep = max(int(k.sum()) for k in keeps)
    VP = max(128, ((nkeep + 127) // 128) * 128)

    in_maps = []
    for b in range(B):
        k = keeps[b]
        in_maps.append(_prep_core(gc[b], R[b], shank[b], logits[b],
                                  v1_pos[k], v1_prf[k], VP))
    nc = _build_nc(VP)
    _split_multiwaits(nc)
    res = run_bass_kernel_spmd(nc, in_maps, list(range(B)), trace=trace)
    out = np.stack([res.results[i]["out"] for i in range(B)])
    return out[:, None, :, :].astype(np.float32), res


def kernel(**inputs) -> np.ndarray:
    out, _ = _run(inputs, trace=False)
    return out


# revision 40
# speedup vs baseline: 1.0127x; 1.0025x over previous
"""Trainium2 Bass kernel for nn_DifferentiableSimulator.

Strategy (8 NeuronCores, B=8): one batch element per core, no collectives.

Host side (cheap, O(V+N)):
  - per-batch probe geometry: rotation, LUT bilinear interp (tiny)
  - per-batch voxel relevance sharding: keep voxels within CUT(10mm) +
    probe-radius of the shank axis segment.  Dropped voxels have weights
    < e^-32 relative to any weight that can influence an output pixel
    (full argument in the repo notes); empirically the output matches the
    dense reference to ~2e-4.  Keeps ~1.3k of 10k voxels.
  - lattice factorization: the 1000 contacts are a rigid 10x10x10 grid,
    so in the rotated frame  d2[n,v] = (x_i-wx_v)^2 + (y_j-wy_v)^2 +
    (z_k-wz_v)^2  with w = R^T (v - grid_center).  The soft-match weight
    matrix factorizes as W[n,v] = Wxy[(ij),v] * Wz[k,v]: only 110 gaussian
    columns per voxel instead of 1000.  Host ships the voxel features
    (fp16 hi/lo pairs so the fp16 matmul is ~fp32-exact: fp16 products are
    exact in the fp32 PSUM accumulator) and the 138 lattice columns.
  - contacts are reindexed m = k*128 + (iy*10+ix)  (28 dummy xy slots per
    z-layer with weight 0) so the per-z-layer weighted sums land exactly
    in contact-chunk layout with no transposes.

Device side (per core), phase 1 -- soft PRF match per 128-voxel chunk:
  one K=17 fp16 matmul -> xy/z gaussian exponents [128v, 138] in PSUM;
  ACT exp -> [Wxy | Wz] fp32; one DVE op forms WzE = Wz x [pol, ecc, 1]
  (broadcast APs); one fp32 matmul accumulates B[128ij, 30] =
  sum_v Wxy^T (Wz*E)  = all weighted sums, already contact-major.

Phase 2 -- splat:
  The reference's per-electrode 256x256 gaussian splat is separable:
  exp(-((gx-cx)^2+(gy-cy)^2)/s^2) = col_factor * row_factor, and
  rot90(map) just swaps/mirrors the centers (row center 255-cx, col
  center cy).  Per-contact params (sin/cos via the hardware Sin spline); row/col factors
  via ACT Square with scale=1/s folded in, one big exp per 4/4/2 chunks;
  20 fp16 matmuls accumulate the 256x256 map; global max via the
  PE-transpose trick; scale; DMA out.

A PE warmup burst runs during startup to coax the HAM clock gate to
2.4 GHz (unreliable on these cores; the kernel is sized to be fast even
at the cold 1.2 GHz PE clock).
"""
import math
from contextlib import ExitStack

import numpy as np

import concourse.bass as bass
import concourse.mybir as mybir
from concourse import tile
from concourse.bass_utils import run_bass_kernel_spmd

# ---- constants (must match the reference) ----
_CMAG_A = 0.75
_CMAG_B = 120.0
_CMAG_K = 17.3
_DEG2RAD = math.pi / 180.0
AMP = 100.0
_SPREAD = math.sqrt(AMP / 675.0)
VIEW_ANGLE = 90.0
MAP_SIZE = 256
SOFT_MATCH_SIGMA = 1.5

B = 8
NCC = 10                  # contact chunks = z-layers
NXY = 128                 # xy-lattice slots per layer (100 real + 28 dummy)
CUT = 10.0
XY_RAD = 1.8 * math.sqrt(2.0)
SE = MAP_SIZE / VIEW_ANGLE
KSIG = _SPREAD / 2.0 * SE
EXP_SCALE = 2.0 / (2.0 * SOFT_MATCH_SIGMA ** 2)   # 2/4.5

# sin(y) ~ y*(c0 + c1 y^2 + ... + c4 y^8) on [-pi, pi]; max err 1.7e-5
SIN_C = (9.99984590e-01, -1.66632589e-01, 8.31238590e-03,
         -1.93162309e-04, 2.17323611e-06)

f32 = mybir.dt.float32
f16 = mybir.dt.float16
i32 = mybir.dt.int32
AF = mybir.ActivationFunctionType
ALU = mybir.AluOpType
PI = math.pi


# ---------------------------------------------------------------- host prep
def _f16s(x):
    hi = np.float16(x)
    lo = np.float16(np.float32(x) - np.float32(hi))
    return hi, lo


def _f16_split(x):
    hi = x.astype(np.float16)
    lo = (x.astype(np.float32) - hi.astype(np.float32)).astype(np.float16)
    return hi.astype(np.float32), lo.astype(np.float32)


def _host_geometry(params, start_loc, surf_dist_lut, alpha_grid, beta_grid):
    params = params.astype(np.float64)
    alpha, beta, offset, shank = (params[:, 0], params[:, 1],
                                  params[:, 2], params[:, 3])
    a = alpha * _DEG2RAD
    b = beta * _DEG2RAD
    ca, sa = np.cos(a), np.sin(a)
    cb, sb = np.cos(b), np.sin(b)
    Bn = params.shape[0]
    Rx = np.zeros((Bn, 3, 3)); Ry = np.zeros((Bn, 3, 3))
    Rx[:, 0, 0] = 1; Rx[:, 1, 1] = ca; Rx[:, 1, 2] = -sa
    Rx[:, 2, 1] = sa; Rx[:, 2, 2] = ca
    Ry[:, 0, 0] = cb; Ry[:, 0, 2] = sb; Ry[:, 1, 1] = 1
    Ry[:, 2, 0] = -sb; Ry[:, 2, 2] = cb
    R = Rx @ Ry
    direction = np.einsum('bij,j->bi', R, np.array([0.0, 0.0, -1.0]))
    direction = direction / np.linalg.norm(direction, axis=-1, keepdims=True)
    lut = surf_dist_lut.astype(np.float64)
    na, nb = lut.shape
    ag, bg = alpha_grid.astype(np.float64), beta_grid.astype(np.float64)
    a_norm = 2.0 * (alpha - ag[0]) / (ag[-1] - ag[0] + 1e-08) - 1.0
    b_norm = 2.0 * (beta - bg[0]) / (bg[-1] - bg[0] + 1e-08) - 1.0
    ai = np.clip((a_norm + 1.0) * 0.5 * (na - 1), 0.0, na - 1.0)
    bi = np.clip((b_norm + 1.0) * 0.5 * (nb - 1), 0.0, nb - 1.0)
    a0 = np.clip(np.floor(ai), 0, na - 1).astype(np.int64)
    b0 = np.clip(np.floor(bi), 0, nb - 1).astype(np.int64)
    a1 = np.minimum(a0 + 1, na - 1)
    b1 = np.minimum(b0 + 1, nb - 1)
    fa = ai - a0
    fb = bi - b0
    v00 = lut[a0, b0]; v01 = lut[a0, b1]; v10 = lut[a1, b0]; v11 = lut[a1, b1]
    surf = (v00 * (1 - fa) * (1 - fb) + v01 * (1 - fa) * fb
            + v10 * fa * (1 - fb) + v11 * fa * fb)
    surf = np.maximum(surf, 1.0)
    penetration = surf - shank / 2.0 - offset
    grid_center = (start_loc.astype(np.float64)[None, :]
                   + direction * penetration[:, None])
    return grid_center, R, direction, shank


def _voxel_keep(v1_pos, grid_center, axis_dir, half_len):
    d = v1_pos.astype(np.float64) - grid_center[None, :]
    t = np.clip(d @ axis_dir, -half_len, half_len)
    dist = np.linalg.norm(d - t[:, None] * axis_dir[None, :], axis=1)
    return dist <= (CUT + XY_RAD + 0.5)


def _prep_core(gc_b, R_b, shank_b, logits_b, v1_pos_k, v1_prf_k, VP):
    """Per-core device input arrays for the lattice-factorized kernel."""
    Vk = v1_pos_k.shape[0]
    w = np.zeros((VP, 3))
    w[:Vk] = (v1_pos_k.astype(np.float64) - gc_b[None, :]) @ R_b
    wf = w.astype(np.float32)
    wh, wl = _f16_split(wf)
    bxy = (-0.5 * (w[:, 0] ** 2 + w[:, 1] ** 2)).astype(np.float32)
    bz = (-0.5 * w[:, 2] ** 2).astype(np.float32)
    bxy[Vk:] = -30000.0
    bz[Vk:] = -30000.0
    bxyh, bxyl = _f16_split(bxy)
    bzh, bzl = _f16_split(bz)
    onesv = np.ones(VP, np.float32)
    vt = np.stack([wh[:, 0], wh[:, 1], wl[:, 0], wl[:, 1], wh[:, 0],
                   wh[:, 1], onesv, onesv, bxyh, bxyl,
                   wh[:, 2], wl[:, 2], wh[:, 2], onesv, onesv, bzh, bzl],
                  axis=0).astype(np.float16)

    xs = np.arange(10) * 0.4 - 1.8
    zs = (np.linspace(0.0, 1.0, 10) - 0.5) * float(shank_b)
    cols = np.zeros((17, NXY + 10), np.float32)
    for ij in range(NXY):
        if ij < 100:
            iy, ix = ij // 10, ij % 10
            x, y = xs[ix], xs[iy]
            xh, xl = _f16s(x)
            yh, yl = _f16s(y)
            axyh, axyl = _f16s(-0.5 * (x * x + y * y))
            cols[0:10, ij] = [xh, yh, xh, yh, xl, yl, axyh, axyl, 1.0, 1.0]
        else:
            cols[6, ij] = -30000.0     # dummy xy slot -> Wxy = 0
            cols[8, ij] = 1.0
    for k in range(10):
        z = zs[k]
        zh, zl = _f16s(z)
        azh, azl = _f16s(-0.5 * z * z)
        cols[10:17, NXY + k] = [zh, zh, zl, azh, azl, 1.0, 1.0]
    rhs = cols.astype(np.float16)

    nch = VP // 128
    e3 = np.zeros((VP, 3), np.float32)
    e3[:Vk, 0] = v1_prf_k[:, 0]
    e3[:Vk, 1] = v1_prf_k[:, 1]
    e3[:Vk, 2] = 1.0
    e3t = np.ascontiguousarray(
        e3.reshape(nch, 128, 3).transpose(1, 0, 2).reshape(128, 3 * nch))

    lgt = np.full((NXY, NCC), -30.0, np.float32)
    iy, ix = np.divmod(np.arange(100), 10)
    for k in range(NCC):
        lgt[:100, k] = logits_b[iy * 100 + ix * 10 + k]
    return {"vt": vt, "rhs": rhs, "e3": e3t,
            "lgt": np.ascontiguousarray(lgt),
            "eye": np.eye(128, dtype=np.float32)}


# ------------------------------------------------------------- device kernel
def _split_multiwaits(nc):
    """This walrus build accepts at most ONE sync wait per instruction.
    Tile emits several.  Engine instruction streams execute in order, so
    moving all but one wait onto single-wait NoOps inserted just before
    the instruction preserves semantics exactly."""
    cnt = 0
    for fn in nc.m.functions:
        for blk in fn.blocks:
            out = []
            for inst in blk.instructions:
                si = inst.sync_info
                if si is not None and si.on_wait is not None \
                        and len(si.on_wait) > 1:
                    waits = list(si.on_wait)
                    for w in waits[:-1]:
                        cnt += 1
                        out.append(mybir.InstNoOp(
                            name=f"WSPLIT-{cnt}",
                            engine=inst.engine,
                            ins=[], outs=[],
                            sync_info=mybir.SyncInfo(on_wait=[w],
                                                     on_update=[]),
                        ))
                    inst.sync_info = mybir.SyncInfo(
                        on_wait=[waits[-1]], on_update=list(si.on_update))
                out.append(inst)
            blk.instructions = out
    return cnt


def _build_nc(VP):
    nch = VP // 128
    NL = NXY + 10    # 138 lattice columns
    nc = bass.Bass()
    vt_d = nc.dram_tensor("vt", [17, VP], f16, kind="ExternalInput")
    rhs_d = nc.dram_tensor("rhs", [17, NL], f16, kind="ExternalInput")
    e3_d = nc.dram_tensor("e3", [128, 3 * nch], f32, kind="ExternalInput")
    lgt_d = nc.dram_tensor("lgt", [NXY, NCC], f32, kind="ExternalInput")
    eye_d = nc.dram_tensor("eye", [128, 128], f32, kind="ExternalInput")
    out_d = nc.dram_tensor("out", [MAP_SIZE, MAP_SIZE], f32,
                           kind="ExternalOutput")

    with ExitStack() as ctx:
        tc = ctx.enter_context(tile.TileContext(nc))
        constp = ctx.enter_context(tc.tile_pool(name="const", bufs=1))
        parm = ctx.enter_context(tc.tile_pool(name="parm", bufs=1))
        work = ctx.enter_context(tc.tile_pool(name="work", bufs=6))
        psA = ctx.enter_context(
            tc.tile_pool(name="psA", bufs=1, space=bass.MemorySpace.PSUM))

        # Warmups first (top scheduler priority): ACT table load + PE HAM
        # burst run during the sem-init + input-DMA window.
        scr = constp.tile([1, 1], f32, tag="scr", name="scr")
        nc.vector.memset(scr[:], 0.0)
        nc.scalar.activation(scr[:], scr[:], AF.Exp, bias=0.0, scale=1.0)
        wrm = constp.tile([128, 512], f16, tag="wrm", name="wrm")
        nc.vector.memset(wrm[:], 0.0)
        wps = psA.tile([128, 512], f32, tag="wps", name="wps")
        for _ in range(12):
            nc.tensor.matmul(wps[:], wrm[:, 0:128], wrm[:],
                             start=True, stop=True, skip_group_check=True)

        vt_t = constp.tile([17, VP], f16, tag="vt", name="vt")
        nc.sync.dma_start(vt_t[:], vt_d[:])
        rhs_t = constp.tile([17, NL], f16, tag="rhs", name="rhs")
        nc.gpsimd.dma_start(rhs_t[:], rhs_d[:])
        e3_t = constp.tile([128, 3 * nch], f32, tag="e3", name="e3")
        nc.scalar.dma_start(e3_t[:], e3_d[:])
        lg_t = constp.tile([NXY, NCC], f32, tag="lgt", name="lgt")
        nc.sync.dma_start(lg_t[:], lgt_d[:])
        eye_t = constp.tile([128, 128], f32, tag="eye", name="eye")
        nc.gpsimd.dma_start(eye_t[:], eye_d[:])

        ii_t = constp.tile([128, MAP_SIZE], i32, tag="ii", name="ii")
        nc.gpsimd.iota(ii_t[:], pattern=[[1, MAP_SIZE]], base=0,
                       channel_multiplier=0)
        iof = constp.tile([128, MAP_SIZE], f32, tag="iof", name="iof")
        nc.vector.tensor_copy(iof[:], ii_t[:])
        ones_t = constp.tile([1, 128], f32, tag="ones", name="ones")
        nc.vector.memset(ones_t[:], 1.0)

        # sigmoid(logits) is independent of phase 1 -- run it early on the
        # otherwise-idle DVE.
        en = parm.tile([128, NCC], f32, tag="en", name="en")
        nc.scalar.activation(en[:], lg_t[:], AF.Exp, bias=0.0, scale=-1.0)
        nc.vector.tensor_scalar_add(en[:], en[:], 1.0)
        pb = parm.tile([128, NCC], f32, tag="pb", name="pb")
        nc.vector.reciprocal(pb[:], en[:])

        # ---------------- phase 1: factorized soft match ----------------
        B_ps = psA.tile([128, 3 * NCC], f32, tag="B", name="B")
        with tc.tile_pool(name="psW", bufs=4,
                          space=bass.MemorySpace.PSUM) as psW:
            for k in range(nch):
                ct = psW.tile([128, NL], f32, tag="cross", name="cross")
                nc.tensor.matmul(ct[:], vt_t[:, k * 128:(k + 1) * 128],
                                 rhs_t[:], start=True, stop=True)
                wx = work.tile([128, NL], f32, tag="wx", name="wx")
                nc.scalar.activation(wx[:], ct[:], AF.Exp,
                                     bias=0.0, scale=EXP_SCALE)
                wze = work.tile([128, 3 * NCC], f32, tag="wze", name="wze")
                e3b = e3_t[:, 3 * k:3 * k + 3] \
                    .rearrange("p (one f) -> p one f", one=1) \
                    .broadcast_to([128, NCC, 3])
                wzb = wx[:, NXY:NL] \
                    .rearrange("p (k one) -> p k one", one=1) \
                    .broadcast_to([128, NCC, 3])
                nc.vector.tensor_tensor(
                    wze[:].rearrange("p (k f) -> p k f", f=3),
                    e3b, wzb, ALU.mult)
                nc.tensor.matmul(B_ps[:], wx[:, 0:NXY], wze[:],
                                 start=(k == 0), stop=(k == nch - 1))

        bsb = parm.tile([128, 3 * NCC], f32, tag="bsb", name="bsb")
        nc.vector.tensor_copy(bsb[:], B_ps[:])
        bs3 = bsb[:].rearrange("p (k f) -> p k f", f=3)

        with tc.tile_pool(name="psM", bufs=1,
                          space=bass.MemorySpace.PSUM) as psM:
            def pt(tag):
                return parm.tile([128, NCC], f32, tag=tag, name=tag)

            # ---------------- per-contact params ----------------
            t0 = pt("t0")
            nc.vector.tensor_scalar_add(t0[:], bs3[:, :, 2], 1e-8)
            rws = pt("rws"); nc.vector.reciprocal(rws[:], t0[:])
            pol = pt("pol")
            nc.vector.tensor_mul(pol[:], bs3[:, :, 0], rws[:])
            ecc = pt("ecc")
            nc.vector.tensor_mul(ecc[:], bs3[:, :, 1], rws[:])

            # t20 = [theta | pi/2 - |theta|]; ACT Sin of both halves gives
            # sin(phi) and cos(phi) (hardware spline, 4-ULP).
            t20 = parm.tile([128, 2 * NCC], f32, tag="t20", name="t20")
            nc.vector.tensor_scalar(t20[:, 0:NCC], pol[:], _DEG2RAD, -PI,
                                    ALU.mult, ALU.add)
            nc.vector.tensor_scalar(t20[:, 0:NCC], t20[:, 0:NCC], PI, -PI,
                                    ALU.min, ALU.max)
            nc.scalar.activation(t20[:, NCC:2 * NCC], t20[:, 0:NCC], AF.Abs)
            nc.vector.tensor_scalar(t20[:, NCC:2 * NCC], t20[:, NCC:2 * NCC],
                                    -1.0, PI / 2.0, ALU.mult, ALU.add)
            sc20 = parm.tile([128, 2 * NCC], f32, tag="sc20", name="sc20")
            nc.scalar.activation(sc20[:], t20[:], AF.Sin)
            # dummy exp right after the Sin: pulls the exp-table reload into
            # the params window instead of stalling ACT before the first
            # phase-2 mega-exp (Square runs fine in the exp set).
            dummy_exp = nc.scalar.activation(scr[:], scr[:], AF.Exp,
                                             bias=0.0, scale=1.0)
            sn = sc20[:, 0:NCC]
            cs = sc20[:, NCC:2 * NCC]

            t1 = pt("t1"); nc.vector.tensor_mul(t1[:], ecc[:], cs)
            t2 = pt("t2"); nc.vector.tensor_mul(t2[:], ecc[:], sn)
            nyc = pt("nyc")
            nc.vector.tensor_scalar(nyc[:], t1[:], -SE, -127.0,
                                    ALU.mult, ALU.add)
            nxc = pt("nxc")
            nc.vector.tensor_scalar(nxc[:], t2[:], SE, -128.0,
                                    ALU.mult, ALU.add)

            q1 = pt("q1"); nc.vector.tensor_scalar_add(q1[:], ecc[:], _CMAG_A)
            r1 = pt("r1"); nc.vector.reciprocal(r1[:], q1[:])
            q2 = pt("q2"); nc.vector.tensor_scalar_add(q2[:], ecc[:], _CMAG_B)
            r2 = pt("r2"); nc.vector.reciprocal(r2[:], q2[:])
            dd = pt("dd"); nc.vector.tensor_sub(dd[:], r1[:], r2[:])
            nc.vector.tensor_scalar(dd[:], dd[:], _CMAG_K, 1e-8,
                                    ALU.mult, ALU.add)
            mi = pt("mi"); nc.vector.reciprocal(mi[:], dd[:])
            ps = pt("ps")
            nc.vector.tensor_scalar(ps[:], mi[:], KSIG, 1.0,
                                    ALU.mult, ALU.max)
            sr = pt("sr"); nc.vector.reciprocal(sr[:], ps[:])
            sbx = pt("sbx"); nc.vector.tensor_mul(sbx[:], sr[:], nxc[:])
            sby = pt("sby"); nc.vector.tensor_mul(sby[:], sr[:], nyc[:])

            val = pt("val")
            nc.vector.tensor_scalar_min(val[:], bs3[:, :, 2], 1.0)
            wc = pt("wc"); nc.vector.tensor_mul(wc[:], pb[:], val[:])

            # ---------------- phase 2: separable splat ----------------
            mp0 = psM.tile([128, MAP_SIZE], f32, tag="map0", name="map0")
            mp1 = psM.tile([128, MAP_SIZE], f32, tag="map1", name="map1")
            GROUPS = [(0, 4), (4, 4), (8, 2)]
            for g0, GRP in GROUPS:
                sq4 = work.tile([128, 4 * 2 * MAP_SIZE], f32, tag="sq4",
                                name="sq4")
                for q in range(GRP):
                    c = g0 + q
                    o = q * 2 * MAP_SIZE
                    nc.scalar.activation(sq4[:, o:o + MAP_SIZE], iof[:],
                                         AF.Square, bias=sby[:, c:c + 1],
                                         scale=sr[:, c:c + 1])
                    xs = work.tile([128, MAP_SIZE], f32, tag="xs", name="xs")
                    nc.vector.tensor_scalar(xs[:], iof[:], sr[:, c:c + 1],
                                            sbx[:, c:c + 1], ALU.mult,
                                            ALU.add)
                    nc.vector.tensor_mul(
                        sq4[:, o + MAP_SIZE:o + 2 * MAP_SIZE], xs[:], xs[:])
                xy8 = work.tile([128, 4 * 2 * MAP_SIZE], f16, tag="xy8",
                                name="xy8")
                nc.scalar.activation(xy8[:, 0:GRP * 2 * MAP_SIZE],
                                     sq4[:, 0:GRP * 2 * MAP_SIZE],
                                     AF.Exp, bias=0.0, scale=-1.0)
                for q in range(GRP):
                    c = g0 + q
                    o = q * 2 * MAP_SIZE
                    yy = work.tile([128, MAP_SIZE], f16, tag="yy", name="yy")
                    nc.vector.tensor_scalar_mul(yy[:], xy8[:, o:o + MAP_SIZE],
                                                wc[:, c:c + 1])
                    xx = xy8[:, o + MAP_SIZE:o + 2 * MAP_SIZE]
                    nc.tensor.matmul(mp0[:], yy[:, 0:128], xx,
                                     start=(c == 0), stop=(c == NCC - 1))
                    nc.tensor.matmul(mp1[:], yy[:, 128:256], xx,
                                     start=(c == 0), stop=(c == NCC - 1))

            # ---------------- normalize + store ----------------
            m0 = parm.tile([128, 1], f32, tag="m0", name="m0")
            nc.vector.reduce_max(m0[:], mp0[:], axis=mybir.AxisListType.X)
            m1 = parm.tile([128, 1], f32, tag="m1", name="m1")
            nc.vector.reduce_max(m1[:], mp1[:], axis=mybir.AxisListType.X)
            mx = parm.tile([128, 1], f32, tag="mx", name="mx")
            nc.vector.tensor_max(mx[:], m0[:], m1[:])
            mt = psM.tile([1, 128], f32, tag="mt", name="mt")
            nc.tensor.transpose(mt[:], mx[:], eye_t[:, :])
            gm = parm.tile([1, 1], f32, tag="gm", name="gm")
            nc.vector.reduce_max(gm[:], mt[:], axis=mybir.AxisListType.X)
            nc.vector.tensor_scalar_add(gm[:], gm[:], 1e-8)
            gi = parm.tile([1, 1], f32, tag="gi", name="gi")
            nc.vector.reciprocal(gi[:], gm[:])
            gb = psM.tile([128, 1], f32, tag="gb", name="gb")
            nc.tensor.matmul(gb[:], ones_t[:], gi[:], start=True, stop=True)
            gs = parm.tile([128, 1], f32, tag="gs", name="gs")
            nc.vector.tensor_copy(gs[:], gb[:])

            o0 = work.tile([128, MAP_SIZE], f32, tag="o0", name="o0")
            nc.vector.tensor_scalar_mul(o0[:], mp0[:], gs[:])
            o1 = work.tile([128, MAP_SIZE], f32, tag="o1", name="o1")
            nc.scalar.activation(o1[:], mp1[:], AF.Copy, scale=gs[:])
            nc.sync.dma_start(out_d[0:128, :], o0[:])
            nc.scalar.dma_start(out_d[128:256, :], o1[:])
    return nc


# ----------------------------------------------------------------- entry
def _run(inputs, trace=False):
    params = np.asarray(inputs["params"], np.float32)
    logits = np.asarray(inputs["electrode_logits"], np.float32)
    v1_pos = np.asarray(inputs["v1_pos"], np.float32)
    v1_prf = np.asarray(inputs["v1_prf"], np.float32)
    start_loc = np.asarray(inputs["start_loc"], np.float32)
    surf_dist_lut = np.asarray(inputs["surf_dist_lut"], np.float32)
    alpha_grid = np.asarray(inputs["alpha_grid"], np.float32)
    beta_grid = np.asarray(inputs["beta_grid"], np.float32)

    gc, R, direction, shank = _host_geometry(
        params, start_loc, surf_dist_lut, alpha_grid, beta_grid)
    keeps = [_voxel_keep(v1_pos, gc[b], R[b, :, 2], shank[b] / 2.0)
             for b in range(B)]
    nkeep = max(int(k.sum()) for k in keeps)
    VP = max(128, ((nkeep + 127) // 128) * 128)

    in_maps = []
    for b in range(B):
        k = keeps[b]
        in_maps.append(_prep_core(gc[b], R[b], shank[b], logits[b],
                                  v1_pos[k], v1_prf[k], VP))
    nc = _build_nc(VP)
    _split_multiwaits(nc)
    res = run_bass_kernel_spmd(nc, in_maps, list(range(B)), trace=trace)
    out = np.stack([res.results[i]["out"] for i in range(B)])
    return out[:, None, :, :].astype(np.float32), res


def kernel(**inputs) -> np.ndarray:
    out, _ = _run(inputs, trace=False)
    return out
